# revision 14
# baseline (speedup 1.0000x reference)
"""Trainium2 Bass kernel for the gnn_message_passing problem.

Math reduction: the reference builds a [8192,8192] zero-diagonal adjacency
W_full from per-node Linear(8191,1) weights, forms state = [x | zeros] and
returns (state @ W_full.T + bias)[:, 7168:][:, ::-1].

Because state is zero outside its first 1024 columns, and only output nodes
7168..8191 are read, the whole computation collapses to

    out[b, k] = sum_c x[b, c] * weights[8191-k, c] + bias[8191-k]

i.e. a [32,1024] x [1024,1024]^T matmul + bias (for rows n >= 7168 and
cols c < 1024 we always have c < n, so W_full[n, c] == weights[n, c]).

Distribution: shard the 1024 output features row-wise across 8 cores
(128 each, tensor parallel); every core holds the replicated x. No
collectives — the host concatenates the 8 output slices.

Per-core Bass kernel (latency-optimized, the problem is tiny):
  - weights and x are cast to bf16 on the host (rel err ~2e-3, far inside
    the 2e-2 gate): halves HBM traffic and doubles PE throughput.
  - all inputs stream on the SP HWDGE queue, small operands first and the
    weight block last, so the PSUM-accumulated matmul chain fires the
    moment the last DMA completes with zero stalls in between.
  - bias add on the vector engine (tensor_scalar_add) instead of the
    scalar activation path — avoids a 1.3us ACT_TABLE_LOAD.
  - three pieces of pre-finalize BIR surgery (dead const-memset removal,
    Pool->SP re-homing of barrier bookkeeping, minimal end block) remove
    ~9us of framework overhead from the measured window; see the helper
    docstrings below for the trace evidence behind each.
"""

import numpy as np
import ml_dtypes

import concourse.bacc as bacc
import concourse.bass as bass
import concourse.mybir as mybir
from concourse.bass_utils import run_bass_kernel_spmd
from concourse.tile import TileContext

NODES = 8192
IN_F = 1024
OUT_F = 1024
B = 32
N_CORES = 8
KPC = OUT_F // N_CORES  # output features per core: 128
NCHUNK = IN_F // 128    # contraction chunks: 8

F32 = mybir.dt.float32
BF16 = mybir.dt.bfloat16

_NC = None
LAST_RESULT = None  # BassKernelResults of the most recent run (for profiling)


def _build_nc():
    nc = bacc.Bacc(None, target_bir_lowering=False)

    # Per-core inputs, pre-packed on host so partition dim is contiguous:
    #   wt[p, n*KPC + k'] = W_eff[core*KPC + k', n*128 + p]   (bf16)
    #   xt[p, n*B   + b ] = x[b, n*128 + p]                   (bf16)
    wt = nc.dram_tensor("wt", [128, NCHUNK * KPC], BF16, kind="ExternalInput")
    xt = nc.dram_tensor("xt", [128, NCHUNK * B], BF16, kind="ExternalInput")
    bi = nc.dram_tensor("bi", [KPC, 1], F32, kind="ExternalInput")
    out = nc.dram_tensor("out", [KPC, B], F32, kind="ExternalOutput")

    with TileContext(nc) as tc:
        with (
            tc.tile_pool(name="sbuf", bufs=1) as pool,
            tc.tile_pool(name="psum", bufs=1, space=bass.MemorySpace.PSUM) as psum_pool,
        ):
            wt_t = pool.tile([128, NCHUNK * KPC], BF16)
            xt_t = pool.tile([128, NCHUNK * B], BF16)
            b_t = pool.tile([KPC, 1], F32)
            o_t = pool.tile([KPC, B], F32)
            ps = psum_pool.tile([KPC, B], F32)

            # All inputs on the SP HWDGE ring (keeps the ACT ring untouched so
            # the runtime's per-ring teardown only covers one ring). Small
            # operands first so they land before the weight block: the PE
            # chain fires the moment the weight DMA completes, with no stall
            # between LDWEIGHTS and the first MATMUL.
            nc.sync.dma_start(b_t[:], bi[:])
            nc.sync.dma_start(xt_t[:], xt[:])
            nc.sync.dma_start(wt_t[:], wt[:])

            for n in range(NCHUNK):
                nc.tensor.matmul(
                    ps[:],
                    wt_t[:, n * KPC : (n + 1) * KPC],  # lhsT [c=128, k'=128]
                    xt_t[:, n * B : (n + 1) * B],      # rhs  [c=128, b=32]
                    start=(n == 0),
                    stop=(n == NCHUNK - 1),
                )

            nc.vector.tensor_scalar_add(o_t[:], ps[:], b_t[:])
            # Two half-partition output DMAs issued concurrently from both
            # HWDGE engines (SP + ACT): descriptor generation is per
            # partition row, so each issue is ~half as long as one full DMA,
            # and the two HBM write receipts overlap.
            nc.sync.dma_start(out[: KPC // 2], o_t[: KPC // 2])
            nc.scalar.dma_start(out[KPC // 2 :], o_t[KPC // 2 :])

    # Drop the framework's const-tile memsets ([128,1] constants 0.0/1.0/...)
    # — nothing in this kernel reads them, and they are the only datapath
    # instructions ahead of the DMA issue, so they both waste GpSimd work and
    # drag the profiled start ~3.5us before any real work.
    blk = nc.m.functions[0].blocks[0]
    for inst in [i for i in blk.instructions if isinstance(i, mybir.InstMemset)]:
        blk.instructions.remove(inst)

    _fold_pool_into_sp(nc)
    _trim_end_block(nc)

    nc.finalize()
    return nc


def _trim_end_block(nc):
    """Reduce Tile's end block to the single load-bearing instruction: the
    wait for the output DMA's completion receipt.

    Rationale, from trace evidence:
    - The runtime postamble resets every user semaphore unconditionally
      (254 sem ids rewritten each execution), so Tile's semaphore
      range-clear and the barrier fencing it are redundant.
    - The runtime postamble itself begins with an all-engine sync barrier
      on reserved sems 0-2 before any reset, so Tile's own final
      all-engine barrier is redundant too — each engine may fall through
      into the postamble as soon as its stream ends, and the resets start
      only once all engines (including SP) have arrived.
    - SP reaches the postamble only after the output-DMA receipt wait, and
      every other engine's work is transitively complete before that (PE ->
      DVE -> out-DMA issue -> receipt), so no user-semaphore traffic is in
      flight when the resets begin.
    - The receipt wait itself MUST stay: dropping it wedged the device
      (NRT_EXEC_UNIT_UNRECOVERABLE) — the ring rearm raced the in-flight
      output write."""
    f = nc.m.functions[0]

    # The output DMA is the DMACopy writing the ExternalOutput dram tensor;
    # its on_update sem is the receipt sem we must keep waiting on.
    out_sems = set()
    for b in f.blocks:
        for i in b.instructions:
            if isinstance(i, mybir.InstDMACopy) and i.sync_info is not None:
                outs = i.outs or []
                names = {getattr(o, "memref", "") or "" for o in outs}
                if any(n.startswith("out") for n in names):
                    out_sems.update(s.id for s in i.sync_info.on_update)
    assert out_sems, "could not locate the output DMA's completion semaphore"

    # Pre-finalize this is a single SP drain waiting on every completion sem
    # (the compiler later splits it into per-sem event waits).
    end_blk = f.blocks[-1]
    insts = end_blk.instructions
    keep = [
        i
        for i in insts
        if i.sync_info is not None
        and any(s.id in out_sems for s in i.sync_info.on_wait)
    ]
    assert keep, "end block lost the output-DMA receipt wait"
    for i in [i for i in insts if i not in keep]:
        insts.remove(i)


def _fold_pool_into_sp(nc):
    """Re-home every Pool (GpSimd) instruction onto the SP engine so the NEFF
    carries no Pool queue — the NRT-injected per-engine preamble/postamble
    (sync barriers + 51-semaphore resets) then covers one engine fewer.

    The kernel itself never uses GpSimd; Pool only carries framework
    bookkeeping: the all-engine-barrier leader units (drain, wait-gather
    evsem, release evsem) and the Tile semaphore range-clear. Semaphore
    protocols are count-based, not engine-identity-based, so executing the
    leader units on SP is equivalent — provided each leader unit is inserted
    between SP's own gather-increment and SP's wait-on-release, which is
    exactly where it is spliced below.
    """
    POOL, SP = mybir.EngineType.Pool, mybir.EngineType.SP

    def _sem_names(inst, field):
        si = inst.sync_info
        if si is None:
            return []
        return [getattr(s, "ant_name", "") or "" for s in getattr(si, field)]

    for b in nc.m.functions[0].blocks:
        insts = b.instructions
        pool = [i for i in insts if i.engine == POOL]
        if not pool:
            continue
        for i in pool:
            insts.remove(i)
        pool = [i for i in pool if not isinstance(i, mybir.InstUnconditionalBranch)]
        for i in pool:
            i.engine = SP

        # Split Pool's stream into leader units, each ending at the evsem
        # that increments the barrier *release* sem.
        groups, cur = [], []
        for i in pool:
            cur.append(i)
            if isinstance(i, mybir.InstEventSemaphore) and any(
                "release" in n for n in _sem_names(i, "on_update")
            ) and not any("gather" in n for n in _sem_names(i, "on_wait")):
                groups.append(cur)
                cur = []
        if cur:
            groups.append(cur)

        # Insert each unit right before SP's matching wait-on-release evsem.
        gi = 0
        for sp_inst in [i for i in insts if i.engine == SP]:
            if gi >= len(groups):
                break
            if isinstance(sp_inst, mybir.InstEventSemaphore) and any(
                "release" in n for n in _sem_names(sp_inst, "on_wait")
            ):
                pos = insts.index(sp_inst)
                insts[pos:pos] = groups[gi]
                gi += 1
        assert gi == len(groups), (
            f"unmatched pool leader groups in block {b.name}: {gi}/{len(groups)}"
        )


def kernel(x: np.ndarray, weights: np.ndarray, bias: np.ndarray) -> np.ndarray:
    global _NC, LAST_RESULT
    if _NC is None:
        _NC = _build_nc()

    x = np.ascontiguousarray(np.asarray(x, dtype=np.float32))
    weights = np.asarray(weights, dtype=np.float32)
    bias = np.asarray(bias, dtype=np.float32)

    # Effective dense weight block and bias (see module docstring).
    w_eff = weights[NODES - OUT_F :, :IN_F][::-1]  # [1024 (k), 1024 (c)]
    b_eff = bias[NODES - OUT_F :][::-1]            # [1024]

    # Pack per-core operands. w_eff[(i,k'),(n,p)] -> wt[i][p, (n,k')]
    wt_all = w_eff.reshape(N_CORES, KPC, NCHUNK, 128).transpose(0, 3, 2, 1)
    wt_all = np.ascontiguousarray(
        wt_all.reshape(N_CORES, 128, NCHUNK * KPC).astype(ml_dtypes.bfloat16)
    )
    # x[b, (n,p)] -> xt[p, (n,b)], replicated
    xt = np.ascontiguousarray(
        x.reshape(B, NCHUNK, 128).transpose(2, 1, 0).reshape(128, NCHUNK * B)
        .astype(ml_dtypes.bfloat16)
    )
    b_all = np.ascontiguousarray(b_eff.reshape(N_CORES, KPC, 1))

    in_maps = [
        {"wt": wt_all[i], "xt": xt, "bi": b_all[i]} for i in range(N_CORES)
    ]
    LAST_RESULT = run_bass_kernel_spmd(_NC, in_maps, list(range(N_CORES)))

    # Gather: core i returns out[k', b] for k = i*KPC + k'.
    out_t = np.concatenate([r["out"] for r in LAST_RESULT.results], axis=0)
    return np.ascontiguousarray(out_t.T)


# revision 15
# speedup vs baseline: 1.0233x; 1.0233x over previous
"""Trainium2 Bass kernel for the gnn_message_passing problem.

Math reduction: the reference builds a [8192,8192] zero-diagonal adjacency
W_full from per-node Linear(8191,1) weights, forms state = [x | zeros] and
returns (state @ W_full.T + bias)[:, 7168:][:, ::-1].

Because state is zero outside its first 1024 columns, and only output nodes
7168..8191 are read, the whole computation collapses to

    out[b, k] = sum_c x[b, c] * weights[8191-k, c] + bias[8191-k]

i.e. a [32,1024] x [1024,1024]^T matmul + bias (for rows n >= 7168 and
cols c < 1024 we always have c < n, so W_full[n, c] == weights[n, c]).

Distribution: shard the 1024 output features row-wise across 8 cores
(128 each, tensor parallel); every core holds the replicated x. No
collectives — the host concatenates the 8 output slices.

Per-core Bass kernel (latency-optimized, the problem is tiny):
  - weights and x are cast to bf16 on the host (rel err ~2e-3, far inside
    the 2e-2 gate): halves HBM traffic and doubles PE throughput.
  - all inputs stream on the SP HWDGE queue, small operands first and the
    weight block last, so the PSUM-accumulated matmul chain fires the
    moment the last DMA completes with zero stalls in between.
  - bias add on the vector engine (tensor_scalar_add) instead of the
    scalar activation path — avoids a 1.3us ACT_TABLE_LOAD.
  - three pieces of pre-finalize BIR surgery (dead const-memset removal,
    Pool->SP re-homing of barrier bookkeeping, minimal end block) remove
    ~9us of framework overhead from the measured window; see the helper
    docstrings below for the trace evidence behind each.
"""

import numpy as np
import ml_dtypes

import concourse.bacc as bacc
import concourse.bass as bass
import concourse.mybir as mybir
from concourse.bass_utils import run_bass_kernel_spmd
from concourse.tile import TileContext

NODES = 8192
IN_F = 1024
OUT_F = 1024
B = 32
N_CORES = 8
KPC = OUT_F // N_CORES  # output features per core: 128
NCHUNK = IN_F // 128    # contraction chunks: 8

F32 = mybir.dt.float32
BF16 = mybir.dt.bfloat16

_NC = None
LAST_RESULT = None  # BassKernelResults of the most recent run (for profiling)


def _build_nc():
    nc = bacc.Bacc(None, target_bir_lowering=False)

    # Per-core inputs, pre-packed on host so partition dim is contiguous:
    #   wt[p, n*KPC + k'] = W_eff[core*KPC + k', n*128 + p]   (bf16)
    #   xt[p, n*B   + b ] = x[b, n*128 + p]                   (bf16)
    wt = nc.dram_tensor("wt", [128, NCHUNK * KPC], BF16, kind="ExternalInput")
    xt = nc.dram_tensor("xt", [128, NCHUNK * B], BF16, kind="ExternalInput")
    bi = nc.dram_tensor("bi", [KPC, 1], F32, kind="ExternalInput")
    out = nc.dram_tensor("out", [KPC, B], F32, kind="ExternalOutput")

    with TileContext(nc) as tc:
        with (
            tc.tile_pool(name="sbuf", bufs=1) as pool,
            tc.tile_pool(name="psum", bufs=1, space=bass.MemorySpace.PSUM) as psum_pool,
        ):
            wt_t = pool.tile([128, NCHUNK * KPC], BF16)
            xt_t = pool.tile([128, NCHUNK * B], BF16)
            b_t = pool.tile([KPC, 1], F32)
            o_t = pool.tile([KPC, B], F32)
            ps = psum_pool.tile([KPC, B], F32)

            # All inputs on the SP HWDGE ring (keeps the ACT ring untouched so
            # the runtime's per-ring teardown only covers one ring). Small
            # operands first so they land before the weight block: the PE
            # chain fires the moment the weight DMA completes, with no stall
            # between LDWEIGHTS and the first MATMUL.
            nc.sync.dma_start(b_t[:], bi[:])
            nc.sync.dma_start(xt_t[:], xt[:])
            nc.sync.dma_start(wt_t[:], wt[:])

            for n in range(NCHUNK):
                nc.tensor.matmul(
                    ps[:],
                    wt_t[:, n * KPC : (n + 1) * KPC],  # lhsT [c=128, k'=128]
                    xt_t[:, n * B : (n + 1) * B],      # rhs  [c=128, b=32]
                    start=(n == 0),
                    stop=(n == NCHUNK - 1),
                )

            nc.vector.tensor_scalar_add(o_t[:], ps[:], b_t[:])
            # Single full-partition output DMA. A/B-tested against two
            # half-partition DMAs issued from both HWDGE engines: the split
            # measured ~220ns SLOWER — both rings feed the same 16 SDMA
            # engines, so the halves serialize while paying double issue
            # overhead.
            nc.sync.dma_start(out[:], o_t[:])

    # Drop the framework's const-tile memsets ([128,1] constants 0.0/1.0/...)
    # — nothing in this kernel reads them, and they are the only datapath
    # instructions ahead of the DMA issue, so they both waste GpSimd work and
    # drag the profiled start ~3.5us before any real work.
    blk = nc.m.functions[0].blocks[0]
    for inst in [i for i in blk.instructions if isinstance(i, mybir.InstMemset)]:
        blk.instructions.remove(inst)

    _fold_pool_into_sp(nc)
    _trim_end_block(nc)

    nc.finalize()
    return nc


def _trim_end_block(nc):
    """Reduce Tile's end block to the single load-bearing instruction: the
    wait for the output DMA's completion receipt.

    Rationale, from trace evidence:
    - The runtime postamble resets every user semaphore unconditionally
      (254 sem ids rewritten each execution), so Tile's semaphore
      range-clear and the barrier fencing it are redundant.
    - The runtime postamble itself begins with an all-engine sync barrier
      on reserved sems 0-2 before any reset, so Tile's own final
      all-engine barrier is redundant too — each engine may fall through
      into the postamble as soon as its stream ends, and the resets start
      only once all engines (including SP) have arrived.
    - SP reaches the postamble only after the output-DMA receipt wait, and
      every other engine's work is transitively complete before that (PE ->
      DVE -> out-DMA issue -> receipt), so no user-semaphore traffic is in
      flight when the resets begin.
    - The receipt wait itself MUST stay: dropping it wedged the device
      (NRT_EXEC_UNIT_UNRECOVERABLE) — the ring rearm raced the in-flight
      output write."""
    f = nc.m.functions[0]

    # The output DMA is the DMACopy writing the ExternalOutput dram tensor;
    # its on_update sem is the receipt sem we must keep waiting on.
    out_sems = set()
    for b in f.blocks:
        for i in b.instructions:
            if isinstance(i, mybir.InstDMACopy) and i.sync_info is not None:
                outs = i.outs or []
                names = {getattr(o, "memref", "") or "" for o in outs}
                if any(n.startswith("out") for n in names):
                    out_sems.update(s.id for s in i.sync_info.on_update)
    assert out_sems, "could not locate the output DMA's completion semaphore"

    # Pre-finalize this is a single SP drain waiting on every completion sem
    # (the compiler later splits it into per-sem event waits).
    end_blk = f.blocks[-1]
    insts = end_blk.instructions
    keep = [
        i
        for i in insts
        if i.sync_info is not None
        and any(s.id in out_sems for s in i.sync_info.on_wait)
    ]
    assert keep, "end block lost the output-DMA receipt wait"
    for i in [i for i in insts if i not in keep]:
        insts.remove(i)


def _fold_pool_into_sp(nc):
    """Re-home every Pool (GpSimd) instruction onto the SP engine so the NEFF
    carries no Pool queue — the NRT-injected per-engine preamble/postamble
    (sync barriers + 51-semaphore resets) then covers one engine fewer.

    The kernel itself never uses GpSimd; Pool only carries framework
    bookkeeping: the all-engine-barrier leader units (drain, wait-gather
    evsem, release evsem) and the Tile semaphore range-clear. Semaphore
    protocols are count-based, not engine-identity-based, so executing the
    leader units on SP is equivalent — provided each leader unit is inserted
    between SP's own gather-increment and SP's wait-on-release, which is
    exactly where it is spliced below.
    """
    POOL, SP = mybir.EngineType.Pool, mybir.EngineType.SP

    def _sem_names(inst, field):
        si = inst.sync_info
        if si is None:
            return []
        return [getattr(s, "ant_name", "") or "" for s in getattr(si, field)]

    for b in nc.m.functions[0].blocks:
        insts = b.instructions
        pool = [i for i in insts if i.engine == POOL]
        if not pool:
            continue
        for i in pool:
            insts.remove(i)
        pool = [i for i in pool if not isinstance(i, mybir.InstUnconditionalBranch)]
        for i in pool:
            i.engine = SP

        # Split Pool's stream into leader units, each ending at the evsem
        # that increments the barrier *release* sem.
        groups, cur = [], []
        for i in pool:
            cur.append(i)
            if isinstance(i, mybir.InstEventSemaphore) and any(
                "release" in n for n in _sem_names(i, "on_update")
            ) and not any("gather" in n for n in _sem_names(i, "on_wait")):
                groups.append(cur)
                cur = []
        if cur:
            groups.append(cur)

        # Insert each unit right before SP's matching wait-on-release evsem.
        gi = 0
        for sp_inst in [i for i in insts if i.engine == SP]:
            if gi >= len(groups):
                break
            if isinstance(sp_inst, mybir.InstEventSemaphore) and any(
                "release" in n for n in _sem_names(sp_inst, "on_wait")
            ):
                pos = insts.index(sp_inst)
                insts[pos:pos] = groups[gi]
                gi += 1
        assert gi == len(groups), (
            f"unmatched pool leader groups in block {b.name}: {gi}/{len(groups)}"
        )


def kernel(x: np.ndarray, weights: np.ndarray, bias: np.ndarray) -> np.ndarray:
    global _NC, LAST_RESULT
    if _NC is None:
        _NC = _build_nc()

    x = np.ascontiguousarray(np.asarray(x, dtype=np.float32))
    weights = np.asarray(weights, dtype=np.float32)
    bias = np.asarray(bias, dtype=np.float32)

    # Effective dense weight block and bias (see module docstring).
    w_eff = weights[NODES - OUT_F :, :IN_F][::-1]  # [1024 (k), 1024 (c)]
    b_eff = bias[NODES - OUT_F :][::-1]            # [1024]

    # Pack per-core operands. w_eff[(i,k'),(n,p)] -> wt[i][p, (n,k')]
    wt_all = w_eff.reshape(N_CORES, KPC, NCHUNK, 128).transpose(0, 3, 2, 1)
    wt_all = np.ascontiguousarray(
        wt_all.reshape(N_CORES, 128, NCHUNK * KPC).astype(ml_dtypes.bfloat16)
    )
    # x[b, (n,p)] -> xt[p, (n,b)], replicated
    xt = np.ascontiguousarray(
        x.reshape(B, NCHUNK, 128).transpose(2, 1, 0).reshape(128, NCHUNK * B)
        .astype(ml_dtypes.bfloat16)
    )
    b_all = np.ascontiguousarray(b_eff.reshape(N_CORES, KPC, 1))

    in_maps = [
        {"wt": wt_all[i], "xt": xt, "bi": b_all[i]} for i in range(N_CORES)
    ]
    LAST_RESULT = run_bass_kernel_spmd(_NC, in_maps, list(range(N_CORES)))

    # Gather: core i returns out[k', b] for k = i*KPC + k'.
    out_t = np.concatenate([r["out"] for r in LAST_RESULT.results], axis=0)
    return np.ascontiguousarray(out_t.T)


# revision 16
# speedup vs baseline: 1.0264x; 1.0031x over previous
"""Trainium2 Bass kernel for the gnn_message_passing problem.

Math reduction: the reference builds a [8192,8192] zero-diagonal adjacency
W_full from per-node Linear(8191,1) weights, forms state = [x | zeros] and
returns (state @ W_full.T + bias)[:, 7168:][:, ::-1].

Because state is zero outside its first 1024 columns, and only output nodes
7168..8191 are read, the whole computation collapses to

    out[b, k] = sum_c x[b, c] * weights[8191-k, c] + bias[8191-k]

i.e. a [32,1024] x [1024,1024]^T matmul + bias (for rows n >= 7168 and
cols c < 1024 we always have c < n, so W_full[n, c] == weights[n, c]).

Distribution: shard the 1024 output features row-wise across 8 cores
(128 each, tensor parallel); every core holds the replicated x. No
collectives — the host concatenates the 8 output slices.

Per-core Bass kernel (latency-optimized, the problem is tiny):
  - weights and x are cast to bf16 on the host (rel err ~2e-3, far inside
    the 2e-2 gate): halves HBM traffic and doubles PE throughput.
  - all inputs stream on the SP HWDGE queue, small operands first and the
    weight block last, so the PSUM-accumulated matmul chain fires the
    moment the last DMA completes with zero stalls in between.
  - bias add on the vector engine (tensor_scalar_add) instead of the
    scalar activation path — avoids a 1.3us ACT_TABLE_LOAD.
  - three pieces of pre-finalize BIR surgery (dead const-memset removal,
    Pool->SP re-homing of barrier bookkeeping, minimal end block) remove
    ~9us of framework overhead from the measured window; see the helper
    docstrings below for the trace evidence behind each.
"""

import numpy as np
import ml_dtypes

import concourse.bacc as bacc
import concourse.bass as bass
import concourse.mybir as mybir
from concourse.bass_utils import run_bass_kernel_spmd
from concourse.tile import TileContext

NODES = 8192
IN_F = 1024
OUT_F = 1024
B = 32
N_CORES = 8
KPC = OUT_F // N_CORES  # output features per core: 128
NCHUNK = IN_F // 128    # contraction chunks: 8

F32 = mybir.dt.float32
BF16 = mybir.dt.bfloat16

_NC = None
LAST_RESULT = None  # BassKernelResults of the most recent run (for profiling)


def _build_nc():
    nc = bacc.Bacc(None, target_bir_lowering=False)

    # Per-core inputs, pre-packed on host so partition dim is contiguous:
    #   wt[p, n*KPC + k'] = W_eff[core*KPC + k', n*128 + p]   (bf16)
    #   xt[p, n*B   + b ] = x[b, n*128 + p]                   (bf16)
    wt = nc.dram_tensor("wt", [128, NCHUNK * KPC], BF16, kind="ExternalInput")
    xt = nc.dram_tensor("xt", [128, NCHUNK * B], BF16, kind="ExternalInput")
    bi = nc.dram_tensor("bi", [KPC, 1], F32, kind="ExternalInput")
    out = nc.dram_tensor("out", [KPC, B], F32, kind="ExternalOutput")

    with TileContext(nc) as tc:
        with (
            tc.tile_pool(name="sbuf", bufs=1) as pool,
            tc.tile_pool(name="psum", bufs=1, space=bass.MemorySpace.PSUM) as psum_pool,
        ):
            wt_t = pool.tile([128, NCHUNK * KPC], BF16)
            xt_t = pool.tile([128, NCHUNK * B], BF16)
            b_t = pool.tile([KPC, 1], F32)
            o_t = pool.tile([KPC, B], F32)
            ps = psum_pool.tile([KPC, B], F32)

            # All inputs on the SP HWDGE ring (keeps the ACT ring untouched so
            # the runtime's per-ring teardown only covers one ring). Small
            # operands first so they land before the weight block: the PE
            # chain fires the moment the weight DMA completes, with no stall
            # between LDWEIGHTS and the first MATMUL.
            nc.sync.dma_start(b_t[:], bi[:])
            nc.sync.dma_start(xt_t[:], xt[:])
            nc.sync.dma_start(wt_t[:], wt[:])
            # Warm-up write: push the (still uninitialized) output tile to HBM
            # while the weight stream is in flight. Costs nothing measured
            # (pre-compute, fully overwritten by the real output DMA below);
            # exercises the SBUF->HBM write + ack path so the final output
            # DMA's completion receipt isn't first-write cold.
            nc.sync.dma_start(out[:], o_t[:])

            for n in range(NCHUNK):
                nc.tensor.matmul(
                    ps[:],
                    wt_t[:, n * KPC : (n + 1) * KPC],  # lhsT [c=128, k'=128]
                    xt_t[:, n * B : (n + 1) * B],      # rhs  [c=128, b=32]
                    start=(n == 0),
                    stop=(n == NCHUNK - 1),
                )

            nc.vector.tensor_scalar_add(o_t[:], ps[:], b_t[:])
            # Single full-partition output DMA. A/B-tested against two
            # half-partition DMAs issued from both HWDGE engines: the split
            # measured ~220ns SLOWER — both rings feed the same 16 SDMA
            # engines, so the halves serialize while paying double issue
            # overhead.
            nc.sync.dma_start(out[:], o_t[:])

    # Drop the framework's const-tile memsets ([128,1] constants 0.0/1.0/...)
    # — nothing in this kernel reads them, and they are the only datapath
    # instructions ahead of the DMA issue, so they both waste GpSimd work and
    # drag the profiled start ~3.5us before any real work.
    blk = nc.m.functions[0].blocks[0]
    for inst in [i for i in blk.instructions if isinstance(i, mybir.InstMemset)]:
        blk.instructions.remove(inst)

    _fold_pool_into_sp(nc)
    _trim_end_block(nc)

    nc.finalize()
    return nc


def _trim_end_block(nc):
    """Reduce Tile's end block to the single load-bearing instruction: the
    wait for the output DMA's completion receipt.

    Rationale, from trace evidence:
    - The runtime postamble resets every user semaphore unconditionally
      (254 sem ids rewritten each execution), so Tile's semaphore
      range-clear and the barrier fencing it are redundant.
    - The runtime postamble itself begins with an all-engine sync barrier
      on reserved sems 0-2 before any reset, so Tile's own final
      all-engine barrier is redundant too — each engine may fall through
      into the postamble as soon as its stream ends, and the resets start
      only once all engines (including SP) have arrived.
    - SP reaches the postamble only after the output-DMA receipt wait, and
      every other engine's work is transitively complete before that (PE ->
      DVE -> out-DMA issue -> receipt), so no user-semaphore traffic is in
      flight when the resets begin.
    - The receipt wait itself MUST stay: dropping it wedged the device
      (NRT_EXEC_UNIT_UNRECOVERABLE) — the ring rearm raced the in-flight
      output write."""
    f = nc.m.functions[0]

    # The output DMA is the DMACopy writing the ExternalOutput dram tensor;
    # its on_update sem is the receipt sem we must keep waiting on.
    out_sems = set()
    for b in f.blocks:
        for i in b.instructions:
            if isinstance(i, mybir.InstDMACopy) and i.sync_info is not None:
                outs = i.outs or []
                names = {getattr(o, "memref", "") or "" for o in outs}
                if any(n.startswith("out") for n in names):
                    out_sems.update(s.id for s in i.sync_info.on_update)
    assert out_sems, "could not locate the output DMA's completion semaphore"

    # Pre-finalize this is a single SP drain waiting on every completion sem
    # (the compiler later splits it into per-sem event waits).
    end_blk = f.blocks[-1]
    insts = end_blk.instructions
    keep = [
        i
        for i in insts
        if i.sync_info is not None
        and any(s.id in out_sems for s in i.sync_info.on_wait)
    ]
    assert keep, "end block lost the output-DMA receipt wait"
    for i in [i for i in insts if i not in keep]:
        insts.remove(i)


def _fold_pool_into_sp(nc):
    """Re-home every Pool (GpSimd) instruction onto the SP engine so the NEFF
    carries no Pool queue — the NRT-injected per-engine preamble/postamble
    (sync barriers + 51-semaphore resets) then covers one engine fewer.

    The kernel itself never uses GpSimd; Pool only carries framework
    bookkeeping: the all-engine-barrier leader units (drain, wait-gather
    evsem, release evsem) and the Tile semaphore range-clear. Semaphore
    protocols are count-based, not engine-identity-based, so executing the
    leader units on SP is equivalent — provided each leader unit is inserted
    between SP's own gather-increment and SP's wait-on-release, which is
    exactly where it is spliced below.
    """
    POOL, SP = mybir.EngineType.Pool, mybir.EngineType.SP

    def _sem_names(inst, field):
        si = inst.sync_info
        if si is None:
            return []
        return [getattr(s, "ant_name", "") or "" for s in getattr(si, field)]

    for b in nc.m.functions[0].blocks:
        insts = b.instructions
        pool = [i for i in insts if i.engine == POOL]
        if not pool:
            continue
        for i in pool:
            insts.remove(i)
        pool = [i for i in pool if not isinstance(i, mybir.InstUnconditionalBranch)]
        for i in pool:
            i.engine = SP

        # Split Pool's stream into leader units, each ending at the evsem
        # that increments the barrier *release* sem.
        groups, cur = [], []
        for i in pool:
            cur.append(i)
            if isinstance(i, mybir.InstEventSemaphore) and any(
                "release" in n for n in _sem_names(i, "on_update")
            ) and not any("gather" in n for n in _sem_names(i, "on_wait")):
                groups.append(cur)
                cur = []
        if cur:
            groups.append(cur)

        # Insert each unit right before SP's matching wait-on-release evsem.
        gi = 0
        for sp_inst in [i for i in insts if i.engine == SP]:
            if gi >= len(groups):
                break
            if isinstance(sp_inst, mybir.InstEventSemaphore) and any(
                "release" in n for n in _sem_names(sp_inst, "on_wait")
            ):
                pos = insts.index(sp_inst)
                insts[pos:pos] = groups[gi]
                gi += 1
        assert gi == len(groups), (
            f"unmatched pool leader groups in block {b.name}: {gi}/{len(groups)}"
        )


def kernel(x: np.ndarray, weights: np.ndarray, bias: np.ndarray) -> np.ndarray:
    global _NC, LAST_RESULT
    if _NC is None:
        _NC = _build_nc()

    x = np.ascontiguousarray(np.asarray(x, dtype=np.float32))
    weights = np.asarray(weights, dtype=np.float32)
    bias = np.asarray(bias, dtype=np.float32)

    # Effective dense weight block and bias (see module docstring).
    w_eff = weights[NODES - OUT_F :, :IN_F][::-1]  # [1024 (k), 1024 (c)]
    b_eff = bias[NODES - OUT_F :][::-1]            # [1024]

    # Pack per-core operands. w_eff[(i,k'),(n,p)] -> wt[i][p, (n,k')]
    wt_all = w_eff.reshape(N_CORES, KPC, NCHUNK, 128).transpose(0, 3, 2, 1)
    wt_all = np.ascontiguousarray(
        wt_all.reshape(N_CORES, 128, NCHUNK * KPC).astype(ml_dtypes.bfloat16)
    )
    # x[b, (n,p)] -> xt[p, (n,b)], replicated
    xt = np.ascontiguousarray(
        x.reshape(B, NCHUNK, 128).transpose(2, 1, 0).reshape(128, NCHUNK * B)
        .astype(ml_dtypes.bfloat16)
    )
    b_all = np.ascontiguousarray(b_eff.reshape(N_CORES, KPC, 1))

    in_maps = [
        {"wt": wt_all[i], "xt": xt, "bi": b_all[i]} for i in range(N_CORES)
    ]
    LAST_RESULT = run_bass_kernel_spmd(_NC, in_maps, list(range(N_CORES)))

    # Gather: core i returns out[k', b] for k = i*KPC + k'.
    out_t = np.concatenate([r["out"] for r in LAST_RESULT.results], axis=0)
    return np.ascontiguousarray(out_t.T)
